# revision 40
# baseline (speedup 1.0000x reference)
"""Multi-head causal self-attention (B=2, S=2048, D=1024, H=16) on 8 TRN2 NeuronCores.

Sharding: data-parallel over batch (2) x tensor-parallel over heads (4 groups of
4 heads). Each core computes Q/K/V projections for its 4 heads, causal
flash-style attention (scores kept transposed [k, q] so no on-chip transposes
are needed), and a partial output projection against its row-slice of W_O.
Host sums the 4 partials per batch and adds the output bias.

All matmul operands are bf16 (hardware streams fp32/fp32r moving operands at
half rate). Scores accumulate in fp32 PSUM; exp runs on ACT into bf16 P tiles.
Softmax denominators come from an extra all-ones column appended to V (the P@V
matmul also produces the row sums); 1/den (reciprocal_approx_fast, SBUF
source) is broadcast across 64 partitions with gpsimd.partition_broadcast.

Schedule: attention q-block groups are software-pipelined (QK of group g+1
issues before P@V of group g) and projection / output-projection chunks are
interleaved as per-group PE fillers, so the PE stays busy while ACT (exp, the
attention-phase critical engine) drains each group's score tiles.
"""

import contextlib
import sys

import numpy as np

sys.path.insert(0, "/opt/trn_rl_repo")

import concourse.bass as bass  # noqa: E402
import concourse.tile as tile  # noqa: E402
from concourse import bacc, mybir  # noqa: E402
from concourse.bass_utils import run_bass_kernel_spmd  # noqa: E402

from ml_dtypes import bfloat16  # noqa: E402

F32 = mybir.dt.float32
BF16 = mybir.dt.bfloat16
AF = mybir.ActivationFunctionType

B, S, D, H = 2, 2048, 1024, 16
DH = D // H          # 64
TPG = 4              # tensor-parallel groups
HPC = H // TPG       # 4 heads per core
CH = HPC * DH        # 256 channels per core
CHA = CH + HPC       # 260: V channels augmented with a ones column per head
NEG = -1.0e9
N_CORES = 8

NQ = S // 512    # 4 q-blocks of 512
NT = S // 128    # 16 s-tiles / k-blocks

_PROG = None  # cached compiled Bass program


def _build_program():
    nc = bacc.Bacc("TRN2", target_bir_lowering=False, debug=False,
                   num_devices=N_CORES)

    xT = nc.dram_tensor("xT", [D, S], BF16, kind="ExternalInput").ap()
    wq = nc.dram_tensor("wq", [D, CH], BF16, kind="ExternalInput").ap()
    wk = nc.dram_tensor("wk", [D, CH], BF16, kind="ExternalInput").ap()
    wv = nc.dram_tensor("wv", [D, CHA], BF16, kind="ExternalInput").ap()
    wo = nc.dram_tensor("wo", [CH, D], BF16, kind="ExternalInput").ap()
    bq = nc.dram_tensor("bq", [128, 2], F32, kind="ExternalInput").ap()
    bk = nc.dram_tensor("bk", [128, 2], F32, kind="ExternalInput").ap()
    bv = nc.dram_tensor("bv", [1, CHA], BF16, kind="ExternalInput").ap()
    tri = nc.dram_tensor("tri", [128, 128], F32, kind="ExternalInput").ap()
    onesf = nc.dram_tensor("onesf", [1, 64], F32, kind="ExternalInput").ap()
    out = nc.dram_tensor("out", [S, D], BF16, kind="ExternalOutput").ap()

    with tile.TileContext(nc) as tc, contextlib.ExitStack() as ctx:
        const = ctx.enter_context(tc.tile_pool(name="const", bufs=1))
        qt = const.tile([128, 2, S], BF16)     # Q^T/8 (+bq/8): chunk m = heads 2m,2m+1
        kt = const.tile([128, 2, S], BF16)     # K^T (+bk)
        va = const.tile([128, NT, CHA], BF16)  # V augmented: [s, head-major 65-col blocks]
        otn = const.tile([128, 2, S], BF16)    # normalized attention out, transposed
        tri_t = const.tile([128, 128], F32)
        ones64f = const.tile([1, 64], F32)
        bq_t = const.tile([128, 2], F32)
        bk_t = const.tile([128, 2], F32)
        wo_t = const.tile([128, 2, D], BF16)
        xt = const.tile([128, 8, S], BF16)
        wq_t = const.tile([128, 8, CH], BF16)
        wk_t = const.tile([128, 8, CH], BF16)
        wv_t = const.tile([128, 8, CHA], BF16)
        # whole-tensor DMAs (one trigger each; packets fan out across the 16
        # SDMA engines regardless) — keeps the issuing engine's stream short
        xTr = xT.rearrange("(a p) s -> p a s", p=128)
        xTc = xT.rearrange("(a p) s -> a p s", p=128)
        wqr = wq.rearrange("(a p) c -> p a c", p=128)
        wqc = wq.rearrange("(a p) c -> a p c", p=128)
        wkr = wk.rearrange("(a p) c -> p a c", p=128)
        wvr = wv.rearrange("(a p) c -> p a c", p=128)
        wor = wo.rearrange("(a p) n -> p a n", p=128)

        # head-critical stream: wq chunk 0 + per-chunk x cols 0:512 first so
        # the first projection's c-accumulation pipelines with the DMA;
        # split across both HW queues to halve the first-MB latency
        nc.scalar.dma_start(wq_t[:, 0, :], wqc[0])
        for c in range(0, 8, 2):
            nc.sync.dma_start(xt[:, c, 0:512], xTc[c][:, 0:512])
            nc.scalar.dma_start(xt[:, c + 1, 0:512], xTc[c + 1][:, 0:512])
        nc.sync.dma_start(wq_t[:, 1:8, :], wqr[:, 1:8, :])
        nc.sync.dma_start(wk_t, wkr)
        nc.sync.dma_start(wv_t, wvr)
        nc.sync.dma_start(bq_t, bq)
        nc.sync.dma_start(bk_t, bk)
        # V bias row broadcast across all 128 partitions (adds the ones
        # column for the denominator trick during the V evacuation)
        bvb = const.tile([128, CHA], BF16)
        nc.sync.dma_start(bvb, bass.AP(
            tensor=bv.tensor, offset=bv.offset,
            ap=[[0, 128]] + list(bv.ap)[1:]))
        nc.sync.dma_start(ones64f, onesf)
        nc.sync.dma_start(tri_t, tri)
        for n in range(1, NQ):
            nc.sync.dma_start(xt[:, :, n * 512:(n + 1) * 512],
                              xTr[:, :, n * 512:(n + 1) * 512])
        nc.sync.dma_start(wo_t, wor)

        # preload the ACT exp table set while ACT is otherwise idle
        nc.scalar.activation(ones64f, ones64f, AF.Exp)

        with tc.tile_pool(name="sm", bufs=4) as sm, \
             tc.tile_pool(name="psp", bufs=2, space="PSUM") as psp:

            def qk_proj(w_t, dst, bias_t, m, n):
                # one 512-col n-chunk of the Q/K projection; DVE evacuation
                # (bias is a per-partition [128,1] tensor-scalar operand)
                ps = psp.tile([128, 512], F32, tag="ps", name="ps")
                for c in range(8):
                    nc.tensor.matmul(
                        ps, (w_t[:, c, m * 128:(m + 1) * 128]),
                        (xt[:, c, n * 512:(n + 1) * 512]),
                        start=(c == 0), stop=(c == 7))
                nc.vector.tensor_scalar_add(
                    dst[:, m, n * 512:(n + 1) * 512], ps, bias_t[:, m:m + 1])

            def v_proj(t):
                # V (not transposed): stationary = x^T tile, moving = wv_aug;
                # bias (with the ones columns) folds into the DVE evacuation
                ps = psp.tile([128, 512], F32, tag="ps", name="ps")
                psv = ps[:, 0:CHA]
                for c in range(8):
                    nc.tensor.matmul(
                        psv, (xt[:, c, t * 128:(t + 1) * 128]),
                        (wv_t[:, c, :]), start=(c == 0), stop=(c == 7))
                nc.vector.tensor_add(va[:, t, :], psv, bvb)

            def out_tile(jj, t):
                # one 128-row tile of the partial output projection; the
                # last q-block runs in the tail where ACT (exp) is done —
                # evacuate there on ACT so DVE's queue (norm muls) can't gate
                so = sm.tile([128, 1024], BF16, tag="so", name="so", bufs=3)
                for n in range(2):
                    ps = psp.tile([128, 512], F32, tag="ps", name="ops")
                    for c2 in range(2):
                        nc.tensor.matmul(
                            ps, (otn[:, c2, t * 128:(t + 1) * 128]),
                            (wo_t[:, c2, n * 512:(n + 1) * 512]),
                            start=(c2 == 0), stop=(c2 == 1))
                    if jj == 3:
                        nc.scalar.copy(so[:, n * 512:(n + 1) * 512], ps)
                    else:
                        nc.vector.tensor_copy(so[:, n * 512:(n + 1) * 512],
                                              ps)
                nc.sync.dma_start(out[t * 128:(t + 1) * 128, :], so)

            pending_norm = [None]  # deferred norm muls of the previous block

            def attention(j, p, fillers):
                # software-pipelined: QK(g+1) issues before P@V(g); one
                # filler (proj / out-proj chunk) per group keeps PE dense
                # while ACT computes exp(g)
                nkb = 4 * (j + 1)       # causal: k-blocks 0..nkb-1
                ngr = nkb // 2
                qsl = slice(j * 512, (j + 1) * 512)
                # the previous block's norm muls emit in our first filler
                # slot: by then its GpSimd broadcast is long done, so the
                # muls never sit in DVE's strict FIFO blocking the filler
                # evacuations the PE needs next (priority inversion)
                if pending_norm[0] is not None:
                    fillers = [pending_norm[0]] + list(fillers)
                    pending_norm[0] = None
                pv = [psp.tile([65, 512], F32, tag="pv", name=f"pv{_hh}")
                      for _hh in range(2)]
                st_g = {}

                def qk_group(g):
                    st = [psp.tile([128, 1024], F32, tag="st",
                                   name=f"st{_hh}") for _hh in range(2)]
                    for i in range(2):
                        kb = 2 * g + i
                        # fully-masked cols [0, rel) are never consumed (the
                        # P@V moving slice skips them) — don't compute them;
                        # exp of the stale PSUM there is discarded
                        lo = max(kb * 128 - j * 512, 0)
                        for hh in range(2):  # packed rows 0-63/64-127
                            oh = hh * 64
                            nc.tensor.matmul(
                                st[hh][:, i * 512 + lo:(i + 1) * 512],
                                (kt[oh:oh + 64, p, kb * 128:(kb + 1) * 128]),
                                (qt[oh:oh + 64, p,
                                    j * 512 + lo:(j + 1) * 512]),
                                start=True, stop=True)
                    st_g[g] = st

                qk_group(0)
                for g in range(ngr):
                    st = st_g.pop(g)
                    for i in range(2):
                        kb = 2 * g + i
                        rel = kb * 128 - j * 512
                        if rel >= 0:
                            # mask only the diagonal 128x128 triangle; fully
                            # masked cols [0, rel) are skipped by the P@V
                            # moving slice instead
                            for hh in range(2):
                                sl = st[hh][:, i * 512 + rel:
                                            i * 512 + rel + 128]
                                nc.vector.tensor_add(sl, sl, tri_t)
                    pt = [None, None]
                    for hh in range(2):
                        pt[hh] = sm.tile([128, 1024], BF16, tag="pt",
                                         name=f"pt{hh}", bufs=6)
                        nc.scalar.activation(pt[hh], st[hh], AF.Exp)
                    if g + 1 < ngr:
                        qk_group(g + 1)
                    if fillers:
                        f = fillers.pop(0)
                        if f is not None:
                            f()
                    for i in range(2):
                        kb = 2 * g + i
                        rel = max(kb * 128 - j * 512, 0)
                        for hh in range(2):
                            h = 2 * p + hh
                            nc.tensor.matmul(
                                pv[hh][:, rel:512],
                                (va[:, kb, h * 65:h * 65 + 65]),
                                (pt[hh][:, i * 512 + rel:(i + 1) * 512]),
                                start=(kb == 0), stop=(kb == nkb - 1),
                                skip_group_check=True)
                for f in fillers:   # drain slots displaced by the prepend
                    if f is not None:
                        f()
                # normalization, hh chains interleaved so the GpSimd
                # broadcasts overlap the DVE ops. pv evacuates to SBUF
                # immediately so the PSUM accumulator frees before the
                # latency-bound chain completes (no WAR stall on next block);
                # reciprocal_approx_fast's BITWISE_NOT seed needs the SBUF
                # copy anyway (PSUM reads corrupt the bit pattern).
                pvs, rec, bcs = [None, None], [None, None], [None, None]
                for hh in range(2):
                    pvs[hh] = sm.tile([64, 512], BF16, tag="pvs", name="pvs")
                    nc.vector.tensor_copy(pvs[hh], pv[hh][0:64, :])
                    den = sm.tile([1, 512], F32, tag="den")
                    nc.vector.tensor_copy(den, pv[hh][64:65, :])
                    rec[hh] = sm.tile([1, 512], F32, tag="rec", name="rec")
                    nc.vector.reciprocal_approx_fast(rec[hh], den)
                    bcs[hh] = sm.tile([64, 512], F32, tag="bcs", name="bcs")
                    nc.gpsimd.partition_broadcast(bcs[hh], rec[hh])
                def _muls(pvs=pvs, bcs=bcs, p=p, qsl=qsl):
                    for hh in range(2):
                        nc.vector.tensor_mul(
                            otn[hh * 64:hh * 64 + 64, p, qsl],
                            pvs[hh], bcs[hh])
                pending_norm[0] = _muls

            P = qk_proj
            V = v_proj
            O = out_tile
            # ---- prologue: minimum inputs for attention(0, 0) ------------
            P(wq_t, qt, bq_t, 0, 0)
            P(wk_t, kt, bk_t, 0, 0)
            for t in range(4):
                V(t)
            # ---- head-pair 0, with remaining projections as fillers ------
            attention(0, 0, [lambda: P(wq_t, qt, bq_t, 0, 1),
                             lambda: P(wk_t, kt, bk_t, 0, 1)])
            V(4)
            V(5)
            attention(1, 0, [lambda: V(6), lambda: V(7),
                             lambda: P(wq_t, qt, bq_t, 0, 2),
                             lambda: P(wk_t, kt, bk_t, 0, 2)])
            attention(2, 0, [lambda: V(8), lambda: V(9),
                             lambda: V(10), lambda: V(11),
                             lambda: P(wq_t, qt, bq_t, 0, 3),
                             lambda: P(wk_t, kt, bk_t, 0, 3)])
            attention(3, 0, [lambda: V(12), lambda: V(13),
                             lambda: V(14), lambda: V(15),
                             lambda: P(wq_t, qt, bq_t, 1, 0),
                             lambda: P(wk_t, kt, bk_t, 1, 0),
                             lambda: P(wq_t, qt, bq_t, 1, 1),
                             lambda: P(wk_t, kt, bk_t, 1, 1)])
            # ---- head-pair 1, with out-projection tiles as fillers -------
            attention(0, 1, [lambda: P(wq_t, qt, bq_t, 1, 2),
                             lambda: P(wk_t, kt, bk_t, 1, 2)])
            attention(1, 1, [lambda: P(wq_t, qt, bq_t, 1, 3),
                             lambda: P(wk_t, kt, bk_t, 1, 3),
                             lambda: O(0, 0), lambda: O(0, 1)])
            attention(2, 1, [lambda: O(0, 2), lambda: O(0, 3),
                             lambda: O(1, 4), lambda: O(1, 5),
                             lambda: O(1, 6), lambda: O(1, 7)])
            attention(3, 1, [None, None, None, None,
                             lambda: O(2, 8), lambda: O(2, 9),
                             lambda: O(2, 10), lambda: O(2, 11)])
            pending_norm[0]()   # flush the last block's norm muls
            pending_norm[0] = None
            for t in range(12, 16):
                O(3, t)

    nc.compile()
    return nc


def _tri_np():
    # within-tile causal triangle: tri[kk, c] = NEG if c < kk else 0
    cs = np.arange(128)[None, :]
    ks = np.arange(128)[:, None]
    return np.where(cs < ks, np.float32(NEG),
                    np.float32(0.0)).astype(np.float32)


def build_in_maps(x, Wq, bq, Wk, bk, Wv, bv, Wo):
    tri_np = _tri_np()
    onesf_np = np.ones((1, 64), dtype=np.float32)
    xT_b = [np.asarray(x[b].T, dtype=np.float32).astype(bfloat16)
            for b in range(B)]
    Wq8 = (np.asarray(Wq, dtype=np.float32) * 0.125)  # fold 1/sqrt(DH) into Q
    in_maps = []
    for c in range(N_CORES):
        b, tp = divmod(c, TPG)
        sl = slice(tp * CH, (tp + 1) * CH)
        wv_aug = np.zeros((D, CHA), dtype=np.float32)
        bv_aug = np.zeros((1, CHA), dtype=np.float32)
        for h in range(HPC):
            hsl = slice(tp * CH + h * DH, tp * CH + (h + 1) * DH)
            wv_aug[:, h * 65:h * 65 + DH] = Wv[:, hsl]
            bv_aug[0, h * 65:h * 65 + DH] = bv[hsl]
            bv_aug[0, h * 65 + DH] = 1.0
        in_maps.append({
            "xT": xT_b[b],
            "wq": Wq8[:, sl].astype(bfloat16),
            "wk": np.asarray(Wk[:, sl], dtype=np.float32).astype(bfloat16),
            "wv": wv_aug.astype(bfloat16),
            "wo": np.asarray(Wo[sl, :], dtype=np.float32).astype(bfloat16),
            "bq": (bq[sl].astype(np.float32) * 0.125).reshape(2, 128).T.copy(),
            "bk": bk[sl].astype(np.float32).reshape(2, 128).T.copy(),
            "bv": bv_aug.astype(bfloat16),
            "tri": tri_np,
            "onesf": onesf_np,
        })
    return in_maps


def _get_program():
    global _PROG
    if _PROG is None:
        _PROG = _build_program()
    return _PROG


def kernel(x, mask, Wq, bq, Wk, bk, Wv, bv, Wo, bo):
    x = np.asarray(x, dtype=np.float32)
    mask = np.asarray(mask)
    Wq, Wk, Wv, Wo = (np.asarray(w, dtype=np.float32)
                      for w in (Wq, Wk, Wv, Wo))
    bq, bk, bv, bo = (np.asarray(b, dtype=np.float32)
                      for b in (bq, bk, bv, bo))
    causal = bool(
        np.array_equal(mask != 0,
                       np.tril(np.ones((S, S), dtype=bool))))
    if not causal:
        # Fallback for non-causal masks: exact host computation.
        q = (x @ Wq + bq).reshape(B, S, H, DH).transpose(0, 2, 1, 3)
        k = (x @ Wk + bk).reshape(B, S, H, DH).transpose(0, 2, 1, 3)
        v = (x @ Wv + bv).reshape(B, S, H, DH).transpose(0, 2, 1, 3)
        attn = np.einsum("bhqd,bhkd->bhqk", q, k) / np.sqrt(np.float32(DH))
        attn = np.where(mask == 0, np.float32(-1e9), attn)
        attn = attn - attn.max(axis=-1, keepdims=True)
        e = np.exp(attn)
        p = e / e.sum(axis=-1, keepdims=True)
        o = np.einsum("bhqk,bhkd->bhqd", p, v)
        o = o.transpose(0, 2, 1, 3).reshape(B, S, D)
        return (o @ Wo + bo).astype(np.float32)

    nc = _get_program()
    in_maps = build_in_maps(x, Wq, bq, Wk, bk, Wv, bv, Wo)
    res = run_bass_kernel_spmd(nc, in_maps, core_ids=list(range(N_CORES)))
    out = np.zeros((B, S, D), dtype=np.float32)
    for c in range(N_CORES):
        out[c // TPG] += res.results[c]["out"].astype(np.float32)
    out += bo.astype(np.float32)
    return out
